# revision 5
# baseline (speedup 1.0000x reference)
"""Trainium2 Bass kernel for nn_MlpMoeBlock (dense MoE, 8 experts).

Reference computation (per token t, hidden size H=1024, ffn M=4096, E=8):
    gates = softmax(x @ wr)                                  # [T, E]
    hid_e = gelu(x @ w1[e] + b1[e])                          # [T, M]
    out_e = hid_e @ w2[e] + b2[e]                            # [T, H]
    y     = sum_e gates[:, e] * out_e                        # [T, H]

Sharding: expert-parallel — core e computes gates (full softmax, its own
expert's column) and its expert's gate-weighted output for ALL 8192 tokens;
the host sums the 8 partial outputs (+ the gate-weighted b2 term, computed
from per-core gate vectors that the kernel also returns).

Per-core layout ("everything transposed", zero on-chip transposes):
  xT  [H, T]  : x transposed on host, streamed in token blocks of TB=512
  fc1 : psum[m_chunk 128, tok 512] = sum_h w1[h,m_chunk].T @ xT[h,tok]
        -> gelu(+b1 per-partition bias) -> hidT in SBUF (float32r)
  fc2 : psum[tok 128, h 512] = sum_m hidT[m, tok_chunk].T @ w2[m, h]
        -> scale by gate column (per-partition scalar) -> out[tok, h]
  router: psum[tok 128, E] = sum_h xT[h,tok_chunk].T @ wr[h, :]
        -> softmax along free dim; expert's gate is column 0 because the
           host rotates wr's columns per core.

The expert's weights (32 MB fp32) don't fit in SBUF, so the M=4096 hidden
dim is split into 2 passes of 2048; each pass keeps its half of w1/w2
resident (16 MB) and streams xT. Big matmuls run as float32r (full PE
speed at moving dim >= 256, ~1e-4 relative error).
"""
import numpy as np
from contextlib import ExitStack

import concourse.bass as bass
import concourse.mybir as mybir
import concourse.tile as tile
from concourse import bacc
from concourse.bass_utils import run_bass_kernel_spmd
from concourse.masks import make_identity

P = 128
T = 8192          # tokens (4*2048)
H = 1024          # hidden
M = 4096          # ffn dim
E = 8             # experts == cores
TB = 512          # token block
NBLK = T // TB    # 16
NPASS = 2
MCPP = (M // P) // NPASS   # m-chunks per pass = 16
HO = H // P       # 8 h-chunks

_CACHED_NC = None


def _build():
    f32 = mybir.dt.float32
    f32r = mybir.dt.float32r
    AF = mybir.ActivationFunctionType

    nc = bacc.Bacc("TRN2", target_bir_lowering=False, debug=False)

    xT = nc.dram_tensor("xT", [H, T], f32r, kind="ExternalInput")
    w1 = nc.dram_tensor("w1", [H, M], f32r, kind="ExternalInput")
    w2 = nc.dram_tensor("w2", [M, H], f32r, kind="ExternalInput")
    b1p = nc.dram_tensor("b1p", [P, M // P], f32, kind="ExternalInput")
    wrp = nc.dram_tensor("wrp", [P, HO * E], f32r, kind="ExternalInput")
    out = nc.dram_tensor("out", [NPASS, T, H], f32, kind="ExternalOutput")
    gout = nc.dram_tensor("gout", [P, T // P], f32, kind="ExternalOutput")

    xT_t = xT.ap().rearrange("(ho hp) t -> hp ho t", hp=P)      # [128, 8, T]
    w1_t = w1.ap().rearrange("(ho hp) m -> hp ho m", hp=P)      # [128, 8, M]
    w2_t = w2.ap().rearrange("(mc mp) h -> mp mc h", mp=P)      # [128, 32, H]
    out_a = out.ap()

    with tile.TileContext(nc) as tc, ExitStack() as ctx:
        const = ctx.enter_context(tc.tile_pool(name="const", bufs=1))
        wpool = ctx.enter_context(tc.tile_pool(name="wpool", bufs=1))
        xpool = ctx.enter_context(tc.tile_pool(name="xpool", bufs=1))
        hpool = ctx.enter_context(tc.tile_pool(name="hpool", bufs=1))
        opool = ctx.enter_context(tc.tile_pool(name="opool", bufs=3))
        spool = ctx.enter_context(tc.tile_pool(name="spool", bufs=2))
        ps_r = ctx.enter_context(tc.tile_pool(name="ps_r", bufs=2, space="PSUM"))
        ps_f1 = ctx.enter_context(tc.tile_pool(name="ps_f1", bufs=2, space="PSUM"))
        ps_f2 = ctx.enter_context(tc.tile_pool(name="ps_f2", bufs=2, space="PSUM"))

        wr_sb = const.tile([P, HO * E], f32r)
        nc.sync.dma_start(wr_sb, wrp.ap())
        b1_sb = const.tile([P, M // P], f32)
        nc.sync.dma_start(b1_sb, b1p.ap())
        ident = const.tile([P, P], f32)
        make_identity(nc, ident)
        g_sb = const.tile([P, T // P], f32)   # this expert's gate, all tokens
        # router logits staging: rows 0:E hold logitsT, rest stay zero
        logit_sb = const.tile([P, TB], f32)
        nc.gpsimd.memset(logit_sb, 0.0)

        for p in range(NPASS):
            # chunked weight loads so compute can start before the full
            # 16 MB half-expert is resident
            w1_sb = wpool.tile([P, HO, P * MCPP], f32r, tag="w1")
            for ho in range(HO):
                nc.sync.dma_start(
                    w1_sb[:, ho, :],
                    w1_t[:, ho, p * P * MCPP:(p + 1) * P * MCPP],
                )
            w2_sb = wpool.tile([P, MCPP, H], f32r, tag="w2")
            for mq in range(4):
                nc.sync.dma_start(
                    w2_sb[:, mq * (MCPP // 4):(mq + 1) * (MCPP // 4), :],
                    w2_t[:, p * MCPP + mq * (MCPP // 4):
                         p * MCPP + (mq + 1) * (MCPP // 4), :],
                )

            for blk in range(NBLK):
                x_sb = xpool.tile([P, HO, TB], f32r, tag="x")
                for xh in range(2):
                    nc.sync.dma_start(
                        x_sb[:, xh * (HO // 2):(xh + 1) * (HO // 2), :],
                        xT_t[:, xh * (HO // 2):(xh + 1) * (HO // 2),
                             blk * TB:(blk + 1) * TB],
                    )

                if p == 0:
                    # router: logitsT[e, tok] with wr stationary (8-col
                    # weight loads), then PE-transpose 128-token chunks
                    # back to token-partition layout for the softmax
                    ps_l = ps_r.tile([E, TB], f32, tag="l")
                    for ho in range(HO):
                        nc.tensor.matmul(
                            ps_l,
                            lhsT=wr_sb[:, ho * E:(ho + 1) * E],
                            rhs=x_sb[:, ho, :],
                            start=(ho == 0),
                            stop=(ho == HO - 1),
                        )
                    nc.scalar.copy(logit_sb[:E, :], ps_l)
                    for c in range(TB // P):
                        col = blk * (TB // P) + c
                        ps = ps_r.tile([P, P], f32, tag="rt")
                        nc.tensor.transpose(
                            ps, logit_sb[:, c * P:(c + 1) * P], ident
                        )
                        negmax = spool.tile([P, 1], f32, tag="negmax")
                        nc.vector.reduce_max(
                            negmax, ps[:, 0:E], axis=mybir.AxisListType.X,
                            negate=True,
                        )
                        expt = spool.tile([P, E], f32, tag="expt")
                        ssum = spool.tile([P, 1], f32, tag="ssum")
                        nc.scalar.activation(
                            expt, ps[:, 0:E], AF.Exp, bias=negmax,
                            accum_out=ssum,
                        )
                        rinv = spool.tile([P, 1], f32, tag="rinv")
                        nc.vector.reciprocal(rinv, ssum)
                        nc.vector.tensor_scalar_mul(
                            g_sb[:, col:col + 1], expt[:, 0:1], rinv
                        )

                # fc1: hidT[m_chunk, tok] = gelu(w1.T @ xT + b1)
                hid_sb = hpool.tile([P, MCPP, TB], f32r, tag="hid")
                for mc in range(MCPP):
                    ps1 = ps_f1.tile([P, TB], f32, tag="h")
                    for ho in range(HO):
                        nc.tensor.matmul(
                            ps1,
                            lhsT=w1_sb[:, ho, mc * P:(mc + 1) * P],
                            rhs=x_sb[:, ho, :],
                            start=(ho == 0),
                            stop=(ho == HO - 1),
                        )
                    bcol = p * MCPP + mc
                    nc.scalar.activation(
                        hid_sb[:, mc, :], ps1, AF.Gelu,
                        bias=b1_sb[:, bcol:bcol + 1],
                    )

                # fc2: out[tok, h] = g * (hidT.T @ w2)
                for hh in range(H // TB):
                    for c in range(TB // P):
                        col = blk * (TB // P) + c
                        ps2 = ps_f2.tile([P, TB], f32, tag="o")
                        for mc in range(MCPP):
                            nc.tensor.matmul(
                                ps2,
                                lhsT=hid_sb[:, mc, c * P:(c + 1) * P],
                                rhs=w2_sb[:, mc, hh * TB:(hh + 1) * TB],
                                start=(mc == 0),
                                stop=(mc == MCPP - 1),
                            )
                        o_sb = opool.tile([P, TB], f32, tag="o")
                        nc.vector.tensor_scalar_mul(
                            o_sb, ps2, g_sb[:, col:col + 1]
                        )
                        t0 = blk * TB + c * P
                        nc.sync.dma_start(
                            out_a[p, t0:t0 + P, hh * TB:(hh + 1) * TB], o_sb
                        )

            if p == 0:
                nc.sync.dma_start(gout.ap(), g_sb)

    nc.compile()
    return nc


def _get_nc():
    global _CACHED_NC
    if _CACHED_NC is None:
        _CACHED_NC = _build()
    return _CACHED_NC


def _make_in_maps(x, w1, b1, w2, wr):
    xT = np.ascontiguousarray(x.reshape(T, H).T).astype(np.float32)
    # pack wr [H, E] -> [128, HO*E], with expert e rotated into column 0
    in_maps = []
    for e in range(E):
        wr_rot = np.concatenate([wr[:, e:], wr[:, :e]], axis=1)  # [H, E]
        wrp = np.ascontiguousarray(
            wr_rot.reshape(HO, P, E).transpose(1, 0, 2).reshape(P, HO * E)
        ).astype(np.float32)
        b1p = np.ascontiguousarray(b1[e].reshape(M // P, P).T).astype(np.float32)
        in_maps.append({
            "xT": xT,
            "w1": np.ascontiguousarray(w1[e]).astype(np.float32),
            "w2": np.ascontiguousarray(w2[e]).astype(np.float32),
            "b1p": b1p,
            "wrp": wrp,
        })
    return in_maps


def _run(x, w1, b1, w2, b2, wr, trace=False, trace_kwargs=None):
    nc = _get_nc()
    in_maps = _make_in_maps(x, w1, b1, w2, wr)
    res = run_bass_kernel_spmd(
        nc, in_maps, core_ids=list(range(E)),
        trace=trace, **(trace_kwargs or {}),
    )
    y = np.zeros((T, H), dtype=np.float64)
    gates = np.empty((E, T), dtype=np.float64)
    for e in range(E):
        o = res.results[e]["out"]
        y += o[0].astype(np.float64) + o[1].astype(np.float64)
        gates[e] = res.results[e]["gout"].T.reshape(T)
    # gate-weighted per-expert fc2 bias, summed over experts on host
    y += gates.T @ b2.astype(np.float64)
    return y.reshape(x.shape).astype(np.float32), res


def kernel(x, w1, b1, w2, b2, wr):
    y, _ = _run(
        np.asarray(x), np.asarray(w1), np.asarray(b1),
        np.asarray(w2), np.asarray(b2), np.asarray(wr),
    )
    return y


# revision 7
# speedup vs baseline: 1.1539x; 1.1539x over previous
"""Trainium2 Bass kernel for nn_MlpMoeBlock (dense MoE, 8 experts).

Reference computation (per token t, hidden size H=1024, ffn M=4096, E=8):
    gates = softmax(x @ wr)                                  # [T, E]
    hid_e = gelu(x @ w1[e] + b1[e])                          # [T, M]
    out_e = hid_e @ w2[e] + b2[e]                            # [T, H]
    y     = sum_e gates[:, e] * out_e                        # [T, H]

Sharding: expert-parallel — core e computes gates (full softmax, its own
expert's column) and its expert's gate-weighted output for ALL 8192 tokens;
the host sums the 8 partial outputs (+ the gate-weighted b2 term, computed
from per-core gate vectors that the kernel also returns).

Per-core layout ("everything transposed", zero on-chip transposes):
  xT  [H, T]  : x transposed on host, streamed in token blocks of TB=512
  fc1 : psum[m_chunk 128, tok 512] = sum_h w1[h,m_chunk].T @ xT[h,tok]
        -> gelu(+b1 per-partition bias) -> hidT in SBUF (float32r)
  fc2 : psum[tok 128, h 512] = sum_m hidT[m, tok_chunk].T @ w2[m, h]
        -> scale by gate column (per-partition scalar) -> out[tok, h]
  router: psum[tok 128, E] = sum_h xT[h,tok_chunk].T @ wr[h, :]
        -> softmax along free dim; expert's gate is column 0 because the
           host rotates wr's columns per core.

The expert's weights (32 MB fp32) don't fit in SBUF, so the M=4096 hidden
dim is split into 2 passes of 2048; each pass keeps its half of w1/w2
resident (16 MB) and streams xT. Big matmuls run as float32r (full PE
speed at moving dim >= 256, ~1e-4 relative error).
"""
import numpy as np
from contextlib import ExitStack

import concourse.bass as bass
import concourse.mybir as mybir
import concourse.tile as tile
from concourse import bacc
from concourse.bass_utils import run_bass_kernel_spmd
from concourse.masks import make_identity

P = 128
T = 8192          # tokens (4*2048)
H = 1024          # hidden
M = 4096          # ffn dim
E = 8             # experts == cores
TB = 512          # token block
NBLK = T // TB    # 16
NPASS = 2
MCPP = (M // P) // NPASS   # m-chunks per pass = 16
HO = H // P       # 8 h-chunks

_CACHED_NC = None


def _build():
    f32 = mybir.dt.float32
    f32r = mybir.dt.float32r
    AF = mybir.ActivationFunctionType

    nc = bacc.Bacc("TRN2", target_bir_lowering=False, debug=False)

    xT = nc.dram_tensor("xT", [H, T], f32r, kind="ExternalInput")
    w1 = nc.dram_tensor("w1", [H, M], f32r, kind="ExternalInput")
    w2 = nc.dram_tensor("w2", [M, H], f32r, kind="ExternalInput")
    b1p = nc.dram_tensor("b1p", [P, M // P], f32, kind="ExternalInput")
    wrp = nc.dram_tensor("wrp", [P, HO * E], f32r, kind="ExternalInput")
    out = nc.dram_tensor("out", [NPASS, T, H], f32, kind="ExternalOutput")
    gout = nc.dram_tensor("gout", [P, T // P], f32, kind="ExternalOutput")

    xT_t = xT.ap().rearrange("(ho hp) t -> hp ho t", hp=P)      # [128, 8, T]
    w1_t = w1.ap().rearrange("(ho hp) m -> hp ho m", hp=P)      # [128, 8, M]
    w2_t = w2.ap().rearrange("(mc mp) h -> mp mc h", mp=P)      # [128, 32, H]
    out_a = out.ap()

    with tile.TileContext(nc) as tc, ExitStack() as ctx:
        const = ctx.enter_context(tc.tile_pool(name="const", bufs=1))
        wpool = ctx.enter_context(tc.tile_pool(name="wpool", bufs=1))
        xpool = ctx.enter_context(tc.tile_pool(name="xpool", bufs=1))
        hpool = ctx.enter_context(tc.tile_pool(name="hpool", bufs=1))
        opool = ctx.enter_context(tc.tile_pool(name="opool", bufs=3))
        spool = ctx.enter_context(tc.tile_pool(name="spool", bufs=2))
        ps_r = ctx.enter_context(tc.tile_pool(name="ps_r", bufs=2, space="PSUM"))
        ps_f1 = ctx.enter_context(tc.tile_pool(name="ps_f1", bufs=2, space="PSUM"))
        ps_f2 = ctx.enter_context(tc.tile_pool(name="ps_f2", bufs=2, space="PSUM"))

        wr_sb = const.tile([P, HO * E], f32r)
        nc.sync.dma_start(wr_sb, wrp.ap())
        b1_sb = const.tile([P, M // P], f32)
        nc.sync.dma_start(b1_sb, b1p.ap())
        ident = const.tile([P, P], f32)
        make_identity(nc, ident)
        g_sb = const.tile([P, T // P], f32)   # this expert's gate, all tokens
        # router logits staging: rows 0:E hold logitsT, rest stay zero
        logit_sb = const.tile([P, TB], f32)
        nc.gpsimd.memset(logit_sb, 0.0)

        for p in range(NPASS):
            # chunked weight loads so compute can start before the full
            # 16 MB half-expert is resident
            w1_sb = wpool.tile([P, HO, P * MCPP], f32r, tag="w1")
            for ho in range(HO):
                nc.sync.dma_start(
                    w1_sb[:, ho, :],
                    w1_t[:, ho, p * P * MCPP:(p + 1) * P * MCPP],
                )
            w2_sb = wpool.tile([P, MCPP, H], f32r, tag="w2")
            for mq in range(4):
                nc.sync.dma_start(
                    w2_sb[:, mq * (MCPP // 4):(mq + 1) * (MCPP // 4), :],
                    w2_t[:, p * MCPP + mq * (MCPP // 4):
                         p * MCPP + (mq + 1) * (MCPP // 4), :],
                )

            for blk in range(NBLK):
                x_sb = xpool.tile([P, HO, TB], f32r, tag="x")
                # x loads go on the gpsimd DMA queue so block 0's tokens
                # don't queue behind the 32 MB of weight loads on sync
                for xh in range(2):
                    nc.gpsimd.dma_start(
                        x_sb[:, xh * (HO // 2):(xh + 1) * (HO // 2), :],
                        xT_t[:, xh * (HO // 2):(xh + 1) * (HO // 2),
                             blk * TB:(blk + 1) * TB],
                    )

                if p == 0:
                    # router: logitsT[e, tok] with wr stationary (8-col
                    # weight loads); the PE transpose back to token-
                    # partition layout happens after fc1, giving the ACT
                    # logits copy time to land without stalling the PE
                    ps_l = ps_r.tile([E, TB], f32, tag="l")
                    for ho in range(HO):
                        nc.tensor.matmul(
                            ps_l,
                            lhsT=wr_sb[:, ho * E:(ho + 1) * E],
                            rhs=x_sb[:, ho, :],
                            start=(ho == 0),
                            stop=(ho == HO - 1),
                        )
                    nc.scalar.copy(logit_sb[:E, :], ps_l)

                # fc1: hidT[m_chunk, tok] = gelu(w1.T @ xT + b1)
                hid_sb = hpool.tile([P, MCPP, TB], f32r, tag="hid")
                for mc in range(MCPP):
                    ps1 = ps_f1.tile([P, TB], f32, tag="h")
                    for ho in range(HO):
                        nc.tensor.matmul(
                            ps1,
                            lhsT=w1_sb[:, ho, mc * P:(mc + 1) * P],
                            rhs=x_sb[:, ho, :],
                            start=(ho == 0),
                            stop=(ho == HO - 1),
                        )
                    bcol = p * MCPP + mc
                    nc.scalar.activation(
                        hid_sb[:, mc, :], ps1, AF.Gelu,
                        bias=b1_sb[:, bcol:bcol + 1],
                    )

                if p == 0:
                    # gates: PE-transpose logit chunks to token-partition
                    # layout, then softmax along the free (expert) dim
                    for c in range(TB // P):
                        col = blk * (TB // P) + c
                        ps = ps_r.tile([P, P], f32, tag="rt")
                        nc.tensor.transpose(
                            ps, logit_sb[:, c * P:(c + 1) * P], ident
                        )
                        negmax = spool.tile([P, 1], f32, tag="negmax")
                        nc.vector.reduce_max(
                            negmax, ps[:, 0:E], axis=mybir.AxisListType.X,
                            negate=True,
                        )
                        expt = spool.tile([P, E], f32, tag="expt")
                        ssum = spool.tile([P, 1], f32, tag="ssum")
                        nc.scalar.activation(
                            expt, ps[:, 0:E], AF.Exp, bias=negmax,
                            accum_out=ssum,
                        )
                        rinv = spool.tile([P, 1], f32, tag="rinv")
                        nc.vector.reciprocal(rinv, ssum)
                        nc.vector.tensor_scalar_mul(
                            g_sb[:, col:col + 1], expt[:, 0:1], rinv
                        )

                # fc2: out[tok, h] = g * (hidT.T @ w2)
                for hh in range(H // TB):
                    for c in range(TB // P):
                        col = blk * (TB // P) + c
                        ps2 = ps_f2.tile([P, TB], f32, tag="o")
                        for mc in range(MCPP):
                            nc.tensor.matmul(
                                ps2,
                                lhsT=hid_sb[:, mc, c * P:(c + 1) * P],
                                rhs=w2_sb[:, mc, hh * TB:(hh + 1) * TB],
                                start=(mc == 0),
                                stop=(mc == MCPP - 1),
                            )
                        o_sb = opool.tile([P, TB], f32, tag="o")
                        nc.vector.tensor_scalar_mul(
                            o_sb, ps2, g_sb[:, col:col + 1]
                        )
                        t0 = blk * TB + c * P
                        nc.sync.dma_start(
                            out_a[p, t0:t0 + P, hh * TB:(hh + 1) * TB], o_sb
                        )

            if p == 0:
                nc.sync.dma_start(gout.ap(), g_sb)

    nc.compile()
    return nc


def _get_nc():
    global _CACHED_NC
    if _CACHED_NC is None:
        _CACHED_NC = _build()
    return _CACHED_NC


def _make_in_maps(x, w1, b1, w2, wr):
    xT = np.ascontiguousarray(x.reshape(T, H).T).astype(np.float32)
    # pack wr [H, E] -> [128, HO*E], with expert e rotated into column 0
    in_maps = []
    for e in range(E):
        wr_rot = np.concatenate([wr[:, e:], wr[:, :e]], axis=1)  # [H, E]
        wrp = np.ascontiguousarray(
            wr_rot.reshape(HO, P, E).transpose(1, 0, 2).reshape(P, HO * E)
        ).astype(np.float32)
        b1p = np.ascontiguousarray(b1[e].reshape(M // P, P).T).astype(np.float32)
        in_maps.append({
            "xT": xT,
            "w1": np.ascontiguousarray(w1[e]).astype(np.float32),
            "w2": np.ascontiguousarray(w2[e]).astype(np.float32),
            "b1p": b1p,
            "wrp": wrp,
        })
    return in_maps


def _run(x, w1, b1, w2, b2, wr, trace=False, trace_kwargs=None):
    nc = _get_nc()
    in_maps = _make_in_maps(x, w1, b1, w2, wr)
    res = run_bass_kernel_spmd(
        nc, in_maps, core_ids=list(range(E)),
        trace=trace, **(trace_kwargs or {}),
    )
    y = np.zeros((T, H), dtype=np.float64)
    gates = np.empty((E, T), dtype=np.float64)
    for e in range(E):
        o = res.results[e]["out"]
        y += o[0].astype(np.float64) + o[1].astype(np.float64)
        gates[e] = res.results[e]["gout"].T.reshape(T)
    # gate-weighted per-expert fc2 bias, summed over experts on host
    y += gates.T @ b2.astype(np.float64)
    return y.reshape(x.shape).astype(np.float32), res


def kernel(x, w1, b1, w2, b2, wr):
    y, _ = _run(
        np.asarray(x), np.asarray(w1), np.asarray(b1),
        np.asarray(w2), np.asarray(b2), np.asarray(wr),
    )
    return y
